# revision 1
# baseline (speedup 1.0000x reference)
"""GRU cell (single timestep) on 8 TRN2 NeuronCores, data-parallel over batch.

Contract: kernel(**inputs) takes FULL numpy inputs (as produced by the
problem's setup_inputs()) and returns the FULL (16384, 1024) float32 output.

Strategy:
  - Shard batch (16384) across 8 cores -> 2048 rows/core. Replicate weights.
  - Host-side packing puts every tensor in feature-major ("transposed world")
    layout so the TensorEngine contraction dim is the partition dim and no
    on-chip transposes are needed:
      xT   [128, 4, 2048]  bf16   [p, k, b] = x[b, 128k+p]
      hT   [128, 8, 2048]  f32    [p, k, b] = hidden[b, 128k+p]
      W**  [128, K, 1024]  bf16   [p, k, o] = W[128k+p, o]   (K=4 for x-side, 8 for h-side)
      bias [128, 24]       f32    [p, 8g+m] = b_g[128m+p]
      outT [128, 8, 2048]  f32    [p, m, b] = out[b, 128m+p]
  - All matmuls in bf16 (PE runs 4x slower on fp32), fp32 PSUM accumulation,
    all elementwise/activations in fp32 with the fp32 hidden state.
"""

import sys

if "/opt/trn_rl_repo" not in sys.path:
    sys.path.insert(0, "/opt/trn_rl_repo")

import numpy as np
import ml_dtypes

import concourse.bass as bass
import concourse.tile as tile
from concourse import bacc, mybir
from concourse.bass_utils import run_bass_kernel_spmd

P = 128
NCORES = 8
BATCH = 16384
NB = BATCH // NCORES          # 2048 rows per core
IN = 512
HID = 1024
KX = IN // P                  # 4
KH = HID // P                 # 8
M = HID // P                  # 8 output-feature chunks
BLK = 512                     # batch columns per block
NBLK = NB // BLK              # 4

F32 = mybir.dt.float32
BF16 = mybir.dt.bfloat16

_CACHE = {}


def _build():
    nc = bacc.Bacc("TRN2", target_bir_lowering=False, debug=False, num_devices=NCORES)

    xT = nc.dram_tensor("xT", [P, KX, NB], BF16, kind="ExternalInput").ap()
    hT = nc.dram_tensor("hT", [P, KH, NB], F32, kind="ExternalInput").ap()
    wxr = nc.dram_tensor("wxr", [P, KX, HID], BF16, kind="ExternalInput").ap()
    wxz = nc.dram_tensor("wxz", [P, KX, HID], BF16, kind="ExternalInput").ap()
    wxh = nc.dram_tensor("wxh", [P, KX, HID], BF16, kind="ExternalInput").ap()
    whr = nc.dram_tensor("whr", [P, KH, HID], BF16, kind="ExternalInput").ap()
    whz = nc.dram_tensor("whz", [P, KH, HID], BF16, kind="ExternalInput").ap()
    whh = nc.dram_tensor("whh", [P, KH, HID], BF16, kind="ExternalInput").ap()
    bias = nc.dram_tensor("bias", [P, 24], F32, kind="ExternalInput").ap()
    outT = nc.dram_tensor("outT", [P, M, NB], F32, kind="ExternalOutput").ap()

    with tile.TileContext(nc) as tc:
        with (
            tc.tile_pool(name="wpool", bufs=1) as wpool,
            tc.tile_pool(name="xpool", bufs=2) as xpool,
            tc.tile_pool(name="hpool", bufs=2) as hpool,
            tc.tile_pool(name="hbbpool", bufs=1) as hbbpool,
            tc.tile_pool(name="rpool", bufs=2) as rpool,
            tc.tile_pool(name="rhpool", bufs=1) as rhpool,
            tc.tile_pool(name="zpool", bufs=1) as zpool,
            tc.tile_pool(name="hcpool", bufs=2) as hcpool,
            tc.tile_pool(name="opool", bufs=3) as opool,
            tc.tile_pool(name="psum", bufs=6, space=bass.MemorySpace.PSUM) as psum,
        ):
            # resident weights + bias
            wxr_s = wpool.tile([P, KX, HID], BF16)
            wxz_s = wpool.tile([P, KX, HID], BF16)
            wxh_s = wpool.tile([P, KX, HID], BF16)
            whr_s = wpool.tile([P, KH, HID], BF16)
            whz_s = wpool.tile([P, KH, HID], BF16)
            whh_s = wpool.tile([P, KH, HID], BF16)
            b_s = wpool.tile([P, 24], F32)
            nc.sync.dma_start(wxr_s[:], wxr[:])
            nc.sync.dma_start(wxz_s[:], wxz[:])
            nc.sync.dma_start(wxh_s[:], wxh[:])
            nc.sync.dma_start(whr_s[:], whr[:])
            nc.sync.dma_start(whz_s[:], whz[:])
            nc.sync.dma_start(whh_s[:], whh[:])
            nc.sync.dma_start(b_s[:], bias[:])

            for blk in range(NBLK):
                sl = bass.ts(blk, BLK)
                xb = xpool.tile([P, KX, BLK], BF16, tag="xb")
                nc.sync.dma_start(xb[:], xT[:, :, sl])
                hb = hpool.tile([P, KH, BLK], F32, tag="hb")
                nc.sync.dma_start(hb[:], hT[:, :, sl])
                hbb = hbbpool.tile([P, KH, BLK], BF16, tag="hbb")
                nc.vector.tensor_copy(hbb[:], hb[:])

                rh = rhpool.tile([P, KH, BLK], BF16, tag="rh")

                # ---- R phase: r = sigmoid(x@Wxr + bxr + h@Whr); rh = r*h
                for m in range(M):
                    ps = psum.tile([P, BLK], F32, tag="ps")
                    mo = bass.ts(m, P)
                    for k in range(KX):
                        nc.tensor.matmul(
                            ps[:], wxr_s[:, k, mo], xb[:, k, :],
                            start=(k == 0), stop=False,
                        )
                    for k in range(KH):
                        nc.tensor.matmul(
                            ps[:], whr_s[:, k, mo], hbb[:, k, :],
                            start=False, stop=(k == KH - 1),
                        )
                    rt = rpool.tile([P, BLK], F32, tag="rt")
                    nc.scalar.activation(
                        rt[:], ps[:], mybir.ActivationFunctionType.Sigmoid,
                        bias=b_s[:, m : m + 1],
                    )
                    nc.vector.tensor_mul(rh[:, m, :], rt[:], hb[:, m, :])

                # ---- Z phase: z = sigmoid(x@Wxz + bxz + h@Whz)
                zf = zpool.tile([P, M, BLK], F32, tag="zf")
                for m in range(M):
                    ps = psum.tile([P, BLK], F32, tag="ps")
                    mo = bass.ts(m, P)
                    for k in range(KX):
                        nc.tensor.matmul(
                            ps[:], wxz_s[:, k, mo], xb[:, k, :],
                            start=(k == 0), stop=False,
                        )
                    for k in range(KH):
                        nc.tensor.matmul(
                            ps[:], whz_s[:, k, mo], hbb[:, k, :],
                            start=False, stop=(k == KH - 1),
                        )
                    nc.scalar.activation(
                        zf[:, m, :], ps[:], mybir.ActivationFunctionType.Sigmoid,
                        bias=b_s[:, 8 + m : 9 + m],
                    )

                # ---- HC phase: hc = tanh(x@Wxh + bxh + rh@Whh); out = hc + z*(h-hc)
                for m in range(M):
                    ps = psum.tile([P, BLK], F32, tag="ps")
                    mo = bass.ts(m, P)
                    for k in range(KX):
                        nc.tensor.matmul(
                            ps[:], wxh_s[:, k, mo], xb[:, k, :],
                            start=(k == 0), stop=False,
                        )
                    for k in range(KH):
                        nc.tensor.matmul(
                            ps[:], whh_s[:, k, mo], rh[:, k, :],
                            start=False, stop=(k == KH - 1),
                        )
                    hct = hcpool.tile([P, BLK], F32, tag="hct")
                    nc.scalar.activation(
                        hct[:], ps[:], mybir.ActivationFunctionType.Tanh,
                        bias=b_s[:, 16 + m : 17 + m],
                    )
                    ot = opool.tile([P, BLK], F32, tag="ot")
                    nc.vector.tensor_sub(ot[:], hb[:, m, :], hct[:])
                    nc.vector.tensor_mul(ot[:], ot[:], zf[:, m, :])
                    nc.vector.tensor_add(ot[:], ot[:], hct[:])
                    nc.sync.dma_start(outT[:, m, sl], ot[:])

    nc.compile()
    return nc


def _pack_feature_major(a: np.ndarray, nchunks: int, dtype) -> np.ndarray:
    # [rows, cols] -> [128, nchunks, cols] with [p, k, c] = a[128k+p, c]
    rows, cols = a.shape
    assert rows == nchunks * P
    return np.ascontiguousarray(
        a.reshape(nchunks, P, cols).transpose(1, 0, 2)
    ).astype(dtype)


def kernel(x, hidden, Wxr, bxr, Whr, Wxz, bxz, Whz, Wxh, bxh, Whh):
    if "nc" not in _CACHE:
        _CACHE["nc"] = _build()
    nc = _CACHE["nc"]

    bf = ml_dtypes.bfloat16
    wxr_p = _pack_feature_major(np.asarray(Wxr, np.float32), KX, bf)
    wxz_p = _pack_feature_major(np.asarray(Wxz, np.float32), KX, bf)
    wxh_p = _pack_feature_major(np.asarray(Wxh, np.float32), KX, bf)
    whr_p = _pack_feature_major(np.asarray(Whr, np.float32), KH, bf)
    whz_p = _pack_feature_major(np.asarray(Whz, np.float32), KH, bf)
    whh_p = _pack_feature_major(np.asarray(Whh, np.float32), KH, bf)
    bias_p = np.ascontiguousarray(
        np.concatenate(
            [
                np.asarray(b, np.float32).reshape(M, P).T
                for b in (bxr, bxz, bxh)
            ],
            axis=1,
        )
    )  # [128, 24]

    x = np.asarray(x, np.float32)
    hidden = np.asarray(hidden, np.float32)

    in_maps = []
    for c in range(NCORES):
        rows = slice(c * NB, (c + 1) * NB)
        xT_p = _pack_feature_major(x[rows].T, KX, bf)        # [128, 4, 2048]
        hT_p = _pack_feature_major(hidden[rows].T, KH, np.float32)
        in_maps.append(
            {
                "xT": xT_p,
                "hT": hT_p,
                "wxr": wxr_p,
                "wxz": wxz_p,
                "wxh": wxh_p,
                "whr": whr_p,
                "whz": whz_p,
                "whh": whh_p,
                "bias": bias_p,
            }
        )

    res = run_bass_kernel_spmd(nc, in_maps, core_ids=list(range(NCORES)))

    out = np.empty((BATCH, HID), np.float32)
    for c in range(NCORES):
        oT = res.results[c]["outT"]  # [128, 8, 2048]
        out[c * NB : (c + 1) * NB] = oT.transpose(1, 0, 2).reshape(HID, NB).T
    return out



# revision 4
# speedup vs baseline: 1.7250x; 1.7250x over previous
"""GRU cell (single timestep) on 8 TRN2 NeuronCores, data-parallel over batch.

Contract: kernel(**inputs) takes FULL numpy inputs (as produced by the
problem's setup_inputs()) and returns the FULL (16384, 1024) float32 output.

Strategy v2 (fp8 DoubleRow):
  - Shard batch (16384) across 8 cores -> 2048 rows/core. Replicate weights.
  - Feature-major ("transposed world") layout so the TensorEngine contraction
    dim is the partition dim:
      acts  [128, K, 2048]  [p, k, b] = act[b, 128k+p]
      W**   [128, K, 1024]  [p, k, o] = W[128k+p, o] * 2048
      bias  [128, 24] f32   [p, 8g+m] = b_g[128m+p]
      outT  [128, 8, 2048]  f32
  - Most matmuls in fp8 e4m3 with perf_mode=DoubleRow (2 contraction rows per
    PE cell per cycle -> ~2x TensorE throughput). All weights pre-scaled by
    2048 so e4m3 weight values are normal-range; the activation instruction
    divides by 2048 (exact power of two). Activations quantized at scale 1.
  - Per-matmul dtype config (CFG below) lets precision-critical paths stay
    bf16. Accumulation is always fp32 in PSUM; gate math in fp32.
  - Batch processed in 4 blocks of 512 columns; block loop is INSIDE the
    k loop so one loaded stationary weight serves 4 matmuls (LDWEIGHTS for
    DoubleRow is slow: 256 cols, no FWL).
"""

import sys

if "/opt/trn_rl_repo" not in sys.path:
    sys.path.insert(0, "/opt/trn_rl_repo")

import numpy as np
import ml_dtypes

import concourse.bass as bass
import concourse.tile as tile
from concourse import bacc, mybir
from concourse.bass_utils import run_bass_kernel_spmd

P = 128
NCORES = 8
BATCH = 16384
NB = BATCH // NCORES          # 2048 rows per core
IN = 512
HID = 1024
KX = IN // P                  # 4
KH = HID // P                 # 8
M = HID // P                  # 8 output-feature chunks
BLK = 512                     # batch columns per block
NBLK = NB // BLK              # 4

F32 = mybir.dt.float32
BF16 = mybir.dt.bfloat16
F8 = mybir.dt.float8e4
DR = mybir.MatmulPerfMode.DoubleRow

SW = 2048.0                   # weight pre-scale (power of two, exact)
INV_SW = 1.0 / SW

# dtype per matmul operand pair: True -> fp8 e4m3 DoubleRow, False -> bf16
CFG = {
    "r_x": True, "r_h": True,
    "z_x": True, "z_h": True,
    "hc_x": False, "hc_h": True,
}

_CACHE = {}


def _gate_matmuls(nc, psl, wx_s, actx, x_f8, wh_s, acth, h_f8, m):
    """Emit all matmuls for one gate, one output chunk m, all 4 batch blocks.

    Block loop is innermost so each stationary weight is streamed 4x.
    """
    mo = bass.ts(m, P)
    bsl = [bass.ts(b, BLK) for b in range(NBLK)]
    if x_f8:
        for t in range(KX // 2):
            for b in range(NBLK):
                nc.tensor.matmul(
                    psl[b][:], wx_s[:, 2 * t : 2 * t + 2, mo],
                    actx[:, 2 * t : 2 * t + 2, bsl[b]],
                    start=(t == 0), stop=False, perf_mode=DR,
                )
    else:
        for k in range(KX):
            for b in range(NBLK):
                nc.tensor.matmul(
                    psl[b][:], wx_s[:, k, mo], actx[:, k, bsl[b]],
                    start=(k == 0), stop=False,
                )
    if h_f8:
        for t in range(KH // 2):
            last = t == KH // 2 - 1
            for b in range(NBLK):
                nc.tensor.matmul(
                    psl[b][:], wh_s[:, 2 * t : 2 * t + 2, mo],
                    acth[:, 2 * t : 2 * t + 2, bsl[b]],
                    start=False, stop=last, perf_mode=DR,
                )
    else:
        for k in range(KH):
            last = k == KH - 1
            for b in range(NBLK):
                nc.tensor.matmul(
                    psl[b][:], wh_s[:, k, mo], acth[:, k, bsl[b]],
                    start=False, stop=last,
                )


def _build(cfg_key):
    nc = bacc.Bacc("TRN2", target_bir_lowering=False, debug=False, num_devices=NCORES)

    need_x8 = CFG["r_x"] or CFG["z_x"] or CFG["hc_x"]
    need_xb = not (CFG["r_x"] and CFG["z_x"] and CFG["hc_x"])

    xdt = {True: F8, False: BF16}
    wdt = xdt

    x8 = nc.dram_tensor("x8", [P, KX, NB], F8, kind="ExternalInput").ap() if need_x8 else None
    xb = nc.dram_tensor("xb", [P, KX, NB], BF16, kind="ExternalInput").ap() if need_xb else None
    h8 = nc.dram_tensor("h8", [P, KH, NB], F8, kind="ExternalInput").ap()
    hb = nc.dram_tensor("hb", [P, KH, NB], BF16, kind="ExternalInput").ap()
    wxr = nc.dram_tensor("wxr", [P, KX, HID], wdt[CFG["r_x"]], kind="ExternalInput").ap()
    wxz = nc.dram_tensor("wxz", [P, KX, HID], wdt[CFG["z_x"]], kind="ExternalInput").ap()
    wxh = nc.dram_tensor("wxh", [P, KX, HID], wdt[CFG["hc_x"]], kind="ExternalInput").ap()
    whr = nc.dram_tensor("whr", [P, KH, HID], wdt[CFG["r_h"]], kind="ExternalInput").ap()
    whz = nc.dram_tensor("whz", [P, KH, HID], wdt[CFG["z_h"]], kind="ExternalInput").ap()
    whh = nc.dram_tensor("whh", [P, KH, HID], wdt[CFG["hc_h"]], kind="ExternalInput").ap()
    bias = nc.dram_tensor("bias", [P, 24], F32, kind="ExternalInput").ap()
    outT = nc.dram_tensor("outT", [P, M, NB], F32, kind="ExternalOutput").ap()

    SIG = mybir.ActivationFunctionType.Sigmoid
    TANH = mybir.ActivationFunctionType.Tanh

    with tile.TileContext(nc) as tc:
        with (
            tc.tile_pool(name="wpool", bufs=1) as wpool,
            tc.tile_pool(name="actpool", bufs=1) as actpool,
            tc.tile_pool(name="rpool", bufs=3) as rpool,
            tc.tile_pool(name="hcpool", bufs=3) as hcpool,
            tc.tile_pool(name="opool", bufs=4) as opool,
            tc.tile_pool(name="psum", bufs=8, space=bass.MemorySpace.PSUM) as psum,
        ):
            bsl = [bass.ts(b, BLK) for b in range(NBLK)]

            # ---- resident tensors; DMA issue order = need order
            b_s = wpool.tile([P, 24], F32)
            nc.sync.dma_start(b_s[:], bias[:])
            wxr_s = wpool.tile([P, KX, HID], wdt[CFG["r_x"]])
            nc.sync.dma_start(wxr_s[:], wxr[:])
            whr_s = wpool.tile([P, KH, HID], wdt[CFG["r_h"]])
            nc.sync.dma_start(whr_s[:], whr[:])

            x8_s = actpool.tile([P, KX, NB], F8, name="x8_s") if need_x8 else None
            h8_s = actpool.tile([P, KH, NB], F8)
            hb_s = actpool.tile([P, KH, NB], BF16)
            xb_s = actpool.tile([P, KX, NB], BF16, name="xb_s") if need_xb else None
            # block-chunked DMAs so the first matmuls only wait on chunk 0
            for b in range(NBLK):
                if need_x8:
                    nc.sync.dma_start(x8_s[:, :, bsl[b]], x8[:, :, bsl[b]])
                nc.sync.dma_start(h8_s[:, :, bsl[b]], h8[:, :, bsl[b]])
            for b in range(NBLK):
                nc.sync.dma_start(hb_s[:, :, bsl[b]], hb[:, :, bsl[b]])

            wxz_s = wpool.tile([P, KX, HID], wdt[CFG["z_x"]])
            nc.sync.dma_start(wxz_s[:], wxz[:])
            whz_s = wpool.tile([P, KH, HID], wdt[CFG["z_h"]])
            nc.sync.dma_start(whz_s[:], whz[:])
            wxh_s = wpool.tile([P, KX, HID], wdt[CFG["hc_x"]])
            nc.sync.dma_start(wxh_s[:], wxh[:])
            whh_s = wpool.tile([P, KH, HID], wdt[CFG["hc_h"]])
            nc.sync.dma_start(whh_s[:], whh[:])
            if need_xb:
                for b in range(NBLK):
                    nc.sync.dma_start(xb_s[:, :, bsl[b]], xb[:, :, bsl[b]])

            rh8_s = actpool.tile([P, KH, NB], F8 if CFG["hc_h"] else BF16)
            zb_s = actpool.tile([P, M, NB], BF16)

            xr = x8_s if CFG["r_x"] else xb_s
            xz = x8_s if CFG["z_x"] else xb_s
            xh = x8_s if CFG["hc_x"] else xb_s

            # ---- Phase R: r = sigmoid((x@Wxr + h@Whr)/SW + bxr); rh = r*h
            for m in range(M):
                psl = [psum.tile([P, BLK], F32, tag="ps", name="ps") for _ in range(NBLK)]
                _gate_matmuls(nc, psl, wxr_s, xr, CFG["r_x"], whr_s, h8_s, CFG["r_h"], m)
                for b in range(NBLK):
                    rt = rpool.tile([P, BLK], F32, tag="rt")
                    nc.scalar.activation(rt[:], psl[b][:], SIG,
                                         bias=b_s[:, m : m + 1], scale=INV_SW)
                    nc.vector.tensor_mul(rh8_s[:, m, bsl[b]], rt[:], hb_s[:, m, bsl[b]])

            # ---- Phase Z: z = sigmoid((x@Wxz + h@Whz)/SW + bxz)
            for m in range(M):
                psl = [psum.tile([P, BLK], F32, tag="ps", name="ps") for _ in range(NBLK)]
                _gate_matmuls(nc, psl, wxz_s, xz, CFG["z_x"], whz_s, h8_s, CFG["z_h"], m)
                for b in range(NBLK):
                    nc.scalar.activation(zb_s[:, m, bsl[b]], psl[b][:], SIG,
                                         bias=b_s[:, 8 + m : 9 + m], scale=INV_SW)

            # ---- Phase HC: hc = tanh((x@Wxh + rh@Whh)/SW + bxh); out = hc + z*(h-hc)
            for m in range(M):
                psl = [psum.tile([P, BLK], F32, tag="ps", name="ps") for _ in range(NBLK)]
                _gate_matmuls(nc, psl, wxh_s, xh, CFG["hc_x"], whh_s, rh8_s, CFG["hc_h"], m)
                for b in range(NBLK):
                    hct = hcpool.tile([P, BLK], F32, tag="hct")
                    nc.scalar.activation(hct[:], psl[b][:], TANH,
                                         bias=b_s[:, 16 + m : 17 + m], scale=INV_SW)
                    ot = opool.tile([P, BLK], F32, tag="ot")
                    nc.vector.tensor_sub(ot[:], hb_s[:, m, bsl[b]], hct[:])
                    nc.vector.tensor_mul(ot[:], ot[:], zb_s[:, m, bsl[b]])
                    nc.vector.tensor_add(ot[:], ot[:], hct[:])
                    nc.sync.dma_start(outT[:, m, bsl[b]], ot[:])

    nc.compile()
    return nc


def _pack_feature_major(a: np.ndarray, nchunks: int, dtype) -> np.ndarray:
    # [rows, cols] -> [128, nchunks, cols] with [p, k, c] = a[128k+p, c]
    rows, cols = a.shape
    assert rows == nchunks * P
    return np.ascontiguousarray(
        a.reshape(nchunks, P, cols).transpose(1, 0, 2)
    ).astype(dtype)


def _pack_inputs(x, hidden, Wxr, bxr, Whr, Wxz, bxz, Whz, Wxh, bxh, Whh):
    f8 = ml_dtypes.float8_e4m3
    bf = ml_dtypes.bfloat16
    wdt = {True: f8, False: bf}

    need_x8 = CFG["r_x"] or CFG["z_x"] or CFG["hc_x"]
    need_xb = not (CFG["r_x"] and CFG["z_x"] and CFG["hc_x"])

    common = {
        "wxr": _pack_feature_major(np.asarray(Wxr, np.float32) * SW, KX, wdt[CFG["r_x"]]),
        "wxz": _pack_feature_major(np.asarray(Wxz, np.float32) * SW, KX, wdt[CFG["z_x"]]),
        "wxh": _pack_feature_major(np.asarray(Wxh, np.float32) * SW, KX, wdt[CFG["hc_x"]]),
        "whr": _pack_feature_major(np.asarray(Whr, np.float32) * SW, KH, wdt[CFG["r_h"]]),
        "whz": _pack_feature_major(np.asarray(Whz, np.float32) * SW, KH, wdt[CFG["z_h"]]),
        "whh": _pack_feature_major(np.asarray(Whh, np.float32) * SW, KH, wdt[CFG["hc_h"]]),
        "bias": np.ascontiguousarray(
            np.concatenate(
                [np.asarray(b, np.float32).reshape(M, P).T for b in (bxr, bxz, bxh)],
                axis=1,
            )
        ),
    }

    x = np.asarray(x, np.float32)
    hidden = np.asarray(hidden, np.float32)
    in_maps = []
    for c in range(NCORES):
        rows = slice(c * NB, (c + 1) * NB)
        xT = x[rows].T
        hT = hidden[rows].T
        m = dict(common)
        if need_x8:
            m["x8"] = _pack_feature_major(xT, KX, f8)
        if need_xb:
            m["xb"] = _pack_feature_major(xT, KX, bf)
        m["h8"] = _pack_feature_major(hT, KH, f8)
        m["hb"] = _pack_feature_major(hT, KH, bf)
        in_maps.append(m)
    return in_maps


def kernel(x, hidden, Wxr, bxr, Whr, Wxz, bxz, Whz, Wxh, bxh, Whh):
    cfg_key = tuple(sorted(CFG.items()))
    if _CACHE.get("cfg") != cfg_key:
        _CACHE.clear()
        _CACHE["nc"] = _build(cfg_key)
        _CACHE["cfg"] = cfg_key
    nc = _CACHE["nc"]

    in_maps = _pack_inputs(x, hidden, Wxr, bxr, Whr, Wxz, bxz, Whz, Wxh, bxh, Whh)
    res = run_bass_kernel_spmd(nc, in_maps, core_ids=list(range(NCORES)))

    out = np.empty((BATCH, HID), np.float32)
    for c in range(NCORES):
        oT = res.results[c]["outT"]  # [128, 8, 2048]
        out[c * NB : (c + 1) * NB] = oT.transpose(1, 0, 2).reshape(HID, NB).T
    return out


# revision 9
# speedup vs baseline: 1.7868x; 1.0359x over previous
"""GRU cell (single timestep) on 8 TRN2 NeuronCores, data-parallel over batch.

Contract: kernel(**inputs) takes FULL numpy inputs (as produced by the
problem's setup_inputs()) and returns the FULL (16384, 1024) float32 output.

Strategy v2 (fp8 DoubleRow):
  - Shard batch (16384) across 8 cores -> 2048 rows/core. Replicate weights.
  - Feature-major ("transposed world") layout so the TensorEngine contraction
    dim is the partition dim:
      acts  [128, K, 2048]  [p, k, b] = act[b, 128k+p]
      W**   [128, K, 1024]  [p, k, o] = W[128k+p, o] * 2048
      bias  [128, 24] f32   [p, 8g+m] = b_g[128m+p]
      outT  [128, 8, 2048]  f32
  - Most matmuls in fp8 e4m3 with perf_mode=DoubleRow (2 contraction rows per
    PE cell per cycle -> ~2x TensorE throughput). All weights pre-scaled by
    2048 so e4m3 weight values are normal-range; the activation instruction
    divides by 2048 (exact power of two). Activations quantized at scale 1.
  - Per-matmul dtype config (CFG below) lets precision-critical paths stay
    bf16. Accumulation is always fp32 in PSUM; gate math in fp32.
  - Batch processed in 4 blocks of 512 columns; block loop is INSIDE the
    k loop so one loaded stationary weight serves 4 matmuls (LDWEIGHTS for
    DoubleRow is slow: 256 cols, no FWL).
"""

import sys

if "/opt/trn_rl_repo" not in sys.path:
    sys.path.insert(0, "/opt/trn_rl_repo")

import numpy as np
import ml_dtypes

import concourse.bass as bass
import concourse.tile as tile
from concourse import bacc, mybir
from concourse.bass_utils import run_bass_kernel_spmd

P = 128
NCORES = 8
BATCH = 16384
NB = BATCH // NCORES          # 2048 rows per core
IN = 512
HID = 1024
KX = IN // P                  # 4
KH = HID // P                 # 8
M = HID // P                  # 8 output-feature chunks
BLK = 512                     # batch columns per block
NBLK = NB // BLK              # 4

F32 = mybir.dt.float32
BF16 = mybir.dt.bfloat16
F8 = mybir.dt.float8e4
DR = mybir.MatmulPerfMode.DoubleRow

SW = 2048.0                   # weight pre-scale (power of two, exact)
INV_SW = 1.0 / SW

# dtype per matmul operand pair: True -> fp8 e4m3 DoubleRow, False -> bf16
CFG = {
    "r_x": True, "r_h": True,
    "z_x": True, "z_h": True,
    "hc_x": False, "hc_h": True,
}

_CACHE = {}


def _gate_matmuls(nc, psl, wx_s, actx, x_f8, wh_s, acth, h_f8, m):
    """Emit all matmuls for one gate, one output chunk m, all 4 batch blocks.

    Block loop is innermost so each stationary weight is streamed 4x.
    """
    mo = bass.ts(m, P)
    bsl = [bass.ts(b, BLK) for b in range(NBLK)]
    if x_f8:
        for t in range(KX // 2):
            for b in range(NBLK):
                nc.tensor.matmul(
                    psl[b][:], wx_s[:, 2 * t : 2 * t + 2, mo],
                    actx[:, 2 * t : 2 * t + 2, bsl[b]],
                    start=(t == 0), stop=False, perf_mode=DR,
                )
    else:
        for k in range(KX):
            for b in range(NBLK):
                nc.tensor.matmul(
                    psl[b][:], wx_s[:, k, mo], actx[:, k, bsl[b]],
                    start=(k == 0), stop=False,
                )
    if h_f8:
        for t in range(KH // 2):
            last = t == KH // 2 - 1
            for b in range(NBLK):
                nc.tensor.matmul(
                    psl[b][:], wh_s[:, 2 * t : 2 * t + 2, mo],
                    acth[:, 2 * t : 2 * t + 2, bsl[b]],
                    start=False, stop=last, perf_mode=DR,
                )
    else:
        for k in range(KH):
            last = k == KH - 1
            for b in range(NBLK):
                nc.tensor.matmul(
                    psl[b][:], wh_s[:, k, mo], acth[:, k, bsl[b]],
                    start=False, stop=last,
                )


def _build(cfg_key):
    nc = bacc.Bacc("TRN2", target_bir_lowering=False, debug=False, num_devices=NCORES)

    need_x8 = CFG["r_x"] or CFG["z_x"] or CFG["hc_x"]
    need_xb = not (CFG["r_x"] and CFG["z_x"] and CFG["hc_x"])

    xdt = {True: F8, False: BF16}
    wdt = xdt

    x8 = nc.dram_tensor("x8", [P, KX, NB], F8, kind="ExternalInput").ap() if need_x8 else None
    xb = nc.dram_tensor("xb", [P, KX, NB], BF16, kind="ExternalInput").ap() if need_xb else None
    h8 = nc.dram_tensor("h8", [P, KH, NB], F8, kind="ExternalInput").ap()
    hb = nc.dram_tensor("hb", [P, KH, NB], BF16, kind="ExternalInput").ap()
    wxr = nc.dram_tensor("wxr", [P, KX, HID], wdt[CFG["r_x"]], kind="ExternalInput").ap()
    wxz = nc.dram_tensor("wxz", [P, KX, HID], wdt[CFG["z_x"]], kind="ExternalInput").ap()
    wxh = nc.dram_tensor("wxh", [P, KX, HID], wdt[CFG["hc_x"]], kind="ExternalInput").ap()
    whr = nc.dram_tensor("whr", [P, KH, HID], wdt[CFG["r_h"]], kind="ExternalInput").ap()
    whz = nc.dram_tensor("whz", [P, KH, HID], wdt[CFG["z_h"]], kind="ExternalInput").ap()
    whh = nc.dram_tensor("whh", [P, KH, HID], wdt[CFG["hc_h"]], kind="ExternalInput").ap()
    bias = nc.dram_tensor("bias", [P, 24], F32, kind="ExternalInput").ap()
    outT = nc.dram_tensor("outT", [P, M, NB], BF16, kind="ExternalOutput").ap()

    SIG = mybir.ActivationFunctionType.Sigmoid
    TANH = mybir.ActivationFunctionType.Tanh

    with tile.TileContext(nc) as tc:
        with (
            tc.tile_pool(name="wpool", bufs=1) as wpool,
            tc.tile_pool(name="actpool", bufs=1) as actpool,
            tc.tile_pool(name="rpool", bufs=3) as rpool,
            tc.tile_pool(name="hcpool", bufs=3) as hcpool,
            tc.tile_pool(name="opool", bufs=4) as opool,
            tc.tile_pool(name="psum", bufs=8, space=bass.MemorySpace.PSUM) as psum,
        ):
            bsl = [bass.ts(b, BLK) for b in range(NBLK)]

            # ---- resident tensors; DMA issue order = need order.
            # First matmul (R phase, m=0, x-side, blk0) gates only on
            # wxr + x8 chunk 0; h-side follows ~1.7us later.
            wxr_s = wpool.tile([P, KX, HID], wdt[CFG["r_x"]])
            nc.sync.dma_start(wxr_s[:], wxr[:])

            x8_s = actpool.tile([P, KX, NB], F8, name="x8_s") if need_x8 else None
            h8_s = actpool.tile([P, KH, NB], F8)
            hb_s = actpool.tile([P, KH, NB], BF16)
            xb_s = actpool.tile([P, KX, NB], BF16, name="xb_s") if need_xb else None
            if need_x8:
                nc.sync.dma_start(x8_s[:, :, bsl[0]], x8[:, :, bsl[0]])
            whr_s = wpool.tile([P, KH, HID], wdt[CFG["r_h"]])
            nc.sync.dma_start(whr_s[:], whr[:])
            nc.sync.dma_start(h8_s[:, :, bsl[0]], h8[:, :, bsl[0]])
            b_s = wpool.tile([P, 24], F32)
            nc.sync.dma_start(b_s[:], bias[:])
            # remaining block chunks
            for b in range(1, NBLK):
                if need_x8:
                    nc.sync.dma_start(x8_s[:, :, bsl[b]], x8[:, :, bsl[b]])
                nc.sync.dma_start(h8_s[:, :, bsl[b]], h8[:, :, bsl[b]])
            for b in range(NBLK):
                nc.sync.dma_start(hb_s[:, :, bsl[b]], hb[:, :, bsl[b]])

            wxz_s = wpool.tile([P, KX, HID], wdt[CFG["z_x"]])
            nc.sync.dma_start(wxz_s[:], wxz[:])
            whz_s = wpool.tile([P, KH, HID], wdt[CFG["z_h"]])
            nc.sync.dma_start(whz_s[:], whz[:])
            wxh_s = wpool.tile([P, KX, HID], wdt[CFG["hc_x"]])
            nc.sync.dma_start(wxh_s[:], wxh[:])
            whh_s = wpool.tile([P, KH, HID], wdt[CFG["hc_h"]])
            nc.sync.dma_start(whh_s[:], whh[:])
            if need_xb:
                for b in range(NBLK):
                    nc.sync.dma_start(xb_s[:, :, bsl[b]], xb[:, :, bsl[b]])

            rh8_s = actpool.tile([P, KH, NB], F8 if CFG["hc_h"] else BF16)
            zb_s = actpool.tile([P, M, NB], BF16)

            xr = x8_s if CFG["r_x"] else xb_s
            xz = x8_s if CFG["z_x"] else xb_s
            xh = x8_s if CFG["hc_x"] else xb_s

            # ---- Phase R: r = sigmoid((x@Wxr + h@Whr)/SW + bxr); rh = r*h
            for m in range(M):
                psl = [psum.tile([P, BLK], F32, tag="ps", name="ps") for _ in range(NBLK)]
                _gate_matmuls(nc, psl, wxr_s, xr, CFG["r_x"], whr_s, h8_s, CFG["r_h"], m)
                for b in range(NBLK):
                    rt = rpool.tile([P, BLK], BF16, tag="rt")
                    nc.scalar.activation(rt[:], psl[b][:], SIG,
                                         bias=b_s[:, m : m + 1], scale=INV_SW)
                    nc.vector.tensor_mul(rh8_s[:, m, bsl[b]], rt[:], hb_s[:, m, bsl[b]])

            # ---- Phase Z: z = sigmoid((x@Wxz + h@Whz)/SW + bxz)
            for m in range(M):
                psl = [psum.tile([P, BLK], F32, tag="ps", name="ps") for _ in range(NBLK)]
                _gate_matmuls(nc, psl, wxz_s, xz, CFG["z_x"], whz_s, h8_s, CFG["z_h"], m)
                for b in range(NBLK):
                    nc.scalar.activation(zb_s[:, m, bsl[b]], psl[b][:], SIG,
                                         bias=b_s[:, 8 + m : 9 + m], scale=INV_SW)

            # ---- Phase HC: hc = tanh((x@Wxh + rh@Whh)/SW + bxh); out = hc + z*(h-hc)
            # All-bf16 elementwise chain (2x DVE throughput); output staged
            # per-m so only 8 big output DMAs are issued.
            for m in range(M):
                psl = [psum.tile([P, BLK], F32, tag="ps", name="ps") for _ in range(NBLK)]
                _gate_matmuls(nc, psl, wxh_s, xh, CFG["hc_x"], whh_s, rh8_s, CFG["hc_h"], m)
                ost = opool.tile([P, NB], BF16, tag="ost")
                for b in range(NBLK):
                    hct = hcpool.tile([P, BLK], BF16, tag="hct")
                    nc.scalar.activation(hct[:], psl[b][:], TANH,
                                         bias=b_s[:, 16 + m : 17 + m], scale=INV_SW)
                    ot = hcpool.tile([P, BLK], BF16, tag="ot")
                    nc.vector.tensor_sub(ot[:], hb_s[:, m, bsl[b]], hct[:])
                    nc.vector.tensor_mul(ot[:], ot[:], zb_s[:, m, bsl[b]])
                    nc.vector.tensor_add(ost[:, bsl[b]], ot[:], hct[:])
                nc.sync.dma_start(outT[:, m, :], ost[:])

    nc.compile()
    return nc


def _pack_feature_major(a: np.ndarray, nchunks: int, dtype) -> np.ndarray:
    # [rows, cols] -> [128, nchunks, cols] with [p, k, c] = a[128k+p, c]
    rows, cols = a.shape
    assert rows == nchunks * P
    return np.ascontiguousarray(
        a.reshape(nchunks, P, cols).transpose(1, 0, 2)
    ).astype(dtype)


def _pack_inputs(x, hidden, Wxr, bxr, Whr, Wxz, bxz, Whz, Wxh, bxh, Whh):
    f8 = ml_dtypes.float8_e4m3
    bf = ml_dtypes.bfloat16
    wdt = {True: f8, False: bf}

    need_x8 = CFG["r_x"] or CFG["z_x"] or CFG["hc_x"]
    need_xb = not (CFG["r_x"] and CFG["z_x"] and CFG["hc_x"])

    common = {
        "wxr": _pack_feature_major(np.asarray(Wxr, np.float32) * SW, KX, wdt[CFG["r_x"]]),
        "wxz": _pack_feature_major(np.asarray(Wxz, np.float32) * SW, KX, wdt[CFG["z_x"]]),
        "wxh": _pack_feature_major(np.asarray(Wxh, np.float32) * SW, KX, wdt[CFG["hc_x"]]),
        "whr": _pack_feature_major(np.asarray(Whr, np.float32) * SW, KH, wdt[CFG["r_h"]]),
        "whz": _pack_feature_major(np.asarray(Whz, np.float32) * SW, KH, wdt[CFG["z_h"]]),
        "whh": _pack_feature_major(np.asarray(Whh, np.float32) * SW, KH, wdt[CFG["hc_h"]]),
        "bias": np.ascontiguousarray(
            np.concatenate(
                [np.asarray(b, np.float32).reshape(M, P).T for b in (bxr, bxz, bxh)],
                axis=1,
            )
        ),
    }

    x = np.asarray(x, np.float32)
    hidden = np.asarray(hidden, np.float32)
    in_maps = []
    for c in range(NCORES):
        rows = slice(c * NB, (c + 1) * NB)
        xT = x[rows].T
        hT = hidden[rows].T
        m = dict(common)
        if need_x8:
            m["x8"] = _pack_feature_major(xT, KX, f8)
        if need_xb:
            m["xb"] = _pack_feature_major(xT, KX, bf)
        m["h8"] = _pack_feature_major(hT, KH, f8)
        m["hb"] = _pack_feature_major(hT, KH, bf)
        in_maps.append(m)
    return in_maps


def kernel(x, hidden, Wxr, bxr, Whr, Wxz, bxz, Whz, Wxh, bxh, Whh):
    cfg_key = tuple(sorted(CFG.items()))
    if _CACHE.get("cfg") != cfg_key:
        _CACHE.clear()
        _CACHE["nc"] = _build(cfg_key)
        _CACHE["cfg"] = cfg_key
    nc = _CACHE["nc"]

    in_maps = _pack_inputs(x, hidden, Wxr, bxr, Whr, Wxz, bxz, Whz, Wxh, bxh, Whh)
    res = run_bass_kernel_spmd(nc, in_maps, core_ids=list(range(NCORES)))

    out = np.empty((BATCH, HID), np.float32)
    for c in range(NCORES):
        oT = np.asarray(res.results[c]["outT"], np.float32)  # [128, 8, 2048] bf16->f32
        out[c * NB : (c + 1) * NB] = oT.transpose(1, 0, 2).reshape(HID, NB).T
    return out
